# revision 5
# baseline (speedup 1.0000x reference)
"""Trainium2 Bass kernel for CustomCombinedLoss (weighted BCE sum + MultiMarginLoss).

loss = -sum(w * (pos_t*log(p) + (1-pos_t)*log(1-p)))          # w=2 for target==0
     + sum_{i: target_i>0} (1/C) * sum_{j != y_i} max(0, margin - x[i,y_i] + x[i,j])

Sharding: pure data parallel over the batch dim, B=16384 rows -> 8 cores x 2048 rows.
Each core computes a partial scalar loss; host sums the 8 partials.

Key optimizations over the f32 baseline (61 us):
  - predictions are downcast to fp16 on the host: halves HBM traffic (the
    bottleneck).  Margin-term error from fp16 quantization is ~1e-5 relative,
    far inside the 2e-2 gate; the BCE side stays f32 end to end.
  - xy = pred[r, y_r] extraction no longer burns a full-tile DVE pass per tile.
    One dma_gather fetches each row's 256B-aligned chunk that contains its
    target element (idx = 16*r + (y>>7), max 32767 fits int16); 16 tiny
    [128,128] scalar_tensor_tensor ops pick the element within the chunk.
  - the hinge runs on DVE in 4x perf mode (fp16 in/out, f32 [P,1] bias and
    f32 accum_out are mode-exempt scalars): ~0.6 us/tile vs 2.05 us on ACT.
  - small input DMAs go through the ACT engine's HWDGE ring so the gather's
    index upload is not queued behind the 8 MB prediction stream.

Per-core layout (rows on partitions, C on the free axis):
  row r = g*128 + p of the shard lives at partition p, tile g (g in 0..15).
  pred DRAM tensor is [T=16, P=128, C=2048] fp16; streamed as 8 supertiles
  of two tiles (1 MiB per DMA).
"""

from contextlib import ExitStack

import numpy as np

import concourse.bacc as bacc
import concourse.bass as bass
import concourse.mybir as mybir
import concourse.tile as tile
from concourse.bass_utils import run_bass_kernel_spmd

WEIGHT = 2.0
MARGIN = 0.5
B, C = 16384, 2048
NCORES = 8
BS = B // NCORES          # rows per core
P = 128                   # partitions
T = BS // P               # row tiles per core
SUPT = 2                  # tiles per streamed supertile
NSUP = T // SUPT
CHUNK = 128               # gathered elements per row (256 bytes of fp16)
F32 = mybir.dt.float32
F16 = mybir.dt.float16
I16 = mybir.dt.int16

AluOp = mybir.AluOpType
ActFn = mybir.ActivationFunctionType
AxisList = mybir.AxisListType


def _loss_program(nc: bass.Bass, tc: "tile.TileContext", pred, pprob, tgt, ymod,
                  idxs, out):
    ctx = ExitStack()
    with ctx:
        const_pool = ctx.enter_context(tc.tile_pool(name="const", bufs=1))
        small_pool = ctx.enter_context(tc.tile_pool(name="small", bufs=1))
        pred_pool = ctx.enter_context(tc.tile_pool(name="pred", bufs=6))

        # ---- small inputs via the ACT HWDGE ring (stays ahead of the stream)
        tgt_t = small_pool.tile([P, T], F32)
        nc.scalar.dma_start(tgt_t[:], tgt[:])
        pprob_t = small_pool.tile([P, T], F32)
        nc.scalar.dma_start(pprob_t[:], pprob[:])
        ymod_t = small_pool.tile([P, T], F32)
        nc.scalar.dma_start(ymod_t[:], ymod[:])
        idxs_t = small_pool.tile([P, CHUNK], I16)
        nc.scalar.dma_start(idxs_t[:], idxs[:])

        # iota 0..127 on every partition, fp16 (integers <= 127 are exact)
        iota7 = const_pool.tile([P, CHUNK], F16)
        nc.gpsimd.iota(
            iota7[:], pattern=[[1, CHUNK]], base=0, channel_multiplier=0,
            allow_small_or_imprecise_dtypes=True,
        )

        # scratch outputs (never read)
        junk16 = const_pool.tile([P, C], F16)
        junk7 = const_pool.tile([P, CHUNK], F16)

        # ---- gather each row's 256B chunk holding pred[r, y_r]
        # chunk for row r = g*128+p lands at ch[p, g*128:(g+1)*128]
        # (split in two: num_idxs=2048 in one dma_gather faults on HW)
        ch = const_pool.tile([P, T * CHUNK], F16)
        rows_view = pred.rearrange("g p (k e) -> (g p k) e", e=CHUNK)
        half = BS // 2
        for h in range(2):
            ch_view = ch[:, h * (T // 2) * CHUNK : (h + 1) * (T // 2) * CHUNK]
            ch_view = ch_view.rearrange("p (g e) -> p g e", e=CHUNK)
            nc.gpsimd.dma_gather(
                ch_view, rows_view,
                idxs_t[:, h * (half // 16) : (h + 1) * (half // 16)],
                num_idxs=half, num_idxs_reg=half, elem_size=CHUNK,
            )

        # xy[p, g] = chunk value at offset ymod = y & 127
        xy_t = small_pool.tile([P, T], F32)
        for g in range(T):
            nc.vector.scalar_tensor_tensor(
                junk7[:], iota7[:], ymod_t[:, g : g + 1],
                ch[:, g * CHUNK : (g + 1) * CHUNK],
                AluOp.is_equal, AluOp.mult, accum_out=xy_t[:, g : g + 1],
            )
        # hinge via tensor_scalar(max, add): relu(x + bias) = max(x, -bias) + bias
        # with bias = margin - xy.  accum_out = reduce(op0_result, op1,
        # initial=scalar2), so scalar2 = C*bias makes accum = sum_j relu(x+bias).
        negb_t = small_pool.tile([P, T], F32)
        nc.vector.tensor_scalar(negb_t[:], xy_t[:], -MARGIN, None, AluOp.add)
        cb_t = small_pool.tile([P, T], F32)
        nc.vector.tensor_scalar(cb_t[:], negb_t[:], -float(C), None, AluOp.mult)

        # ---- BCE row terms (all [P, T] f32, off the critical path)
        # y = max(tgt - 1, 0); pos_t = min(tgt, 1)
        pos_t = small_pool.tile([P, T], F32)
        nc.vector.tensor_scalar(pos_t[:], tgt_t[:], 1.0, None, AluOp.min)
        q_t = small_pool.tile([P, T], F32)
        nc.vector.tensor_scalar(q_t[:], pprob_t[:], -1.0, 1.0, AluOp.mult, AluOp.add)
        lp_t = small_pool.tile([P, T], F32)
        nc.scalar.activation(lp_t[:], pprob_t[:], ActFn.Ln)
        lq_t = small_pool.tile([P, T], F32)
        nc.scalar.activation(lq_t[:], q_t[:], ActFn.Ln)
        nc.vector.tensor_scalar(lp_t[:], lp_t[:], -100.0, None, AluOp.max)
        nc.vector.tensor_scalar(lq_t[:], lq_t[:], -100.0, None, AluOp.max)

        # row_total = pos_t*(acc/C - lp - MARGIN/C) + (2*pos_t - 2)*lq
        # lp2 = lp + MARGIN/C;  d = (2*pos_t - 2)*lq
        lp2_t = small_pool.tile([P, T], F32)
        nc.vector.tensor_scalar(lp2_t[:], lp_t[:], MARGIN / C, None, AluOp.add)
        c2_t = small_pool.tile([P, T], F32)
        nc.vector.tensor_scalar(c2_t[:], pos_t[:], 2.0, -2.0, AluOp.mult, AluOp.add)
        d_t = small_pool.tile([P, T], F32)
        nc.vector.tensor_mul(d_t[:], c2_t[:], lq_t[:])
        inv_c_t = small_pool.tile([P, 1], F32)
        nc.vector.memset(inv_c_t[:], 1.0 / C)
        ones_t = small_pool.tile([P, 1], F32)
        nc.vector.memset(ones_t[:], 1.0)

        # ---- stream predictions, hinge on DVE in 4x mode
        # acc[p, g] = sum_j relu(pred[r, j] + (margin - xy[r]))   (incl. j==y
        # term, which is exactly margin; folded out via lp2 above)
        acc_t = small_pool.tile([P, T], F32)
        for s in range(NSUP):
            st = pred_pool.tile([P, SUPT * C], F16, tag="pred")
            st_view = st[:].rearrange("p (g c) -> p g c", g=SUPT)
            nc.sync.dma_start(st_view, pred[s * SUPT : (s + 1) * SUPT].rearrange(
                "g p c -> p g c"))
            for b in range(SUPT):
                g = s * SUPT + b
                nc.vector.tensor_scalar(
                    junk16[:], st[:, b * C : (b + 1) * C], negb_t[:, g : g + 1],
                    cb_t[:, g : g + 1], AluOp.max, AluOp.add,
                    accum_out=acc_t[:, g : g + 1],
                )

        # ---- epilogue: a = acc/C - lp2;  rowred = sum_g(pos_t*a + d)
        rowred = small_pool.tile([P, 1], F32)
        a_t = small_pool.tile([P, T], F32)
        nc.vector.scalar_tensor_tensor(
            a_t[:], acc_t[:], inv_c_t[:, 0:1], lp2_t[:],
            AluOp.mult, AluOp.subtract,
        )
        b_t = small_pool.tile([P, T], F32)
        nc.vector.tensor_mul(b_t[:], pos_t[:], a_t[:])
        e_t = small_pool.tile([P, T], F32)
        nc.vector.tensor_add(e_t[:], b_t[:], d_t[:])
        nc.vector.reduce_sum(rowred[:], e_t[:], axis=AxisList.X)
        # cross-partition sum via PE: ones[128,1].T @ rowred[128,1] -> [1,1]
        psum_pool = ctx.enter_context(tc.tile_pool(name="psum", bufs=1, space="PSUM"))
        total_ps = psum_pool.tile([1, 1], F32)
        nc.tensor.matmul(total_ps[:], rowred[:], ones_t[:], start=True, stop=True)
        total = small_pool.tile([1, 1], F32)
        nc.vector.tensor_copy(total[:], total_ps[:])
        nc.sync.dma_start(out[:], total[:])


def build_nc() -> bass.Bass:
    nc = bacc.Bacc("TRN2", target_bir_lowering=False, debug=False, num_devices=NCORES)
    pred = nc.dram_tensor("pred", [T, P, C], F16, kind="ExternalInput").ap()
    pprob = nc.dram_tensor("pprob", [P, T], F32, kind="ExternalInput").ap()
    tgt = nc.dram_tensor("tgt", [P, T], F32, kind="ExternalInput").ap()
    ymod = nc.dram_tensor("ymod", [P, T], F32, kind="ExternalInput").ap()
    idxs = nc.dram_tensor("idxs", [P, CHUNK], I16, kind="ExternalInput").ap()
    out = nc.dram_tensor("out", [1, 1], F32, kind="ExternalOutput").ap()
    with tile.TileContext(nc) as tc:
        _loss_program(nc, tc, pred, pprob, tgt, ymod, idxs, out)
    nc.compile()
    return nc


def make_in_maps(positive_prob, predictions, target):
    """Shard full inputs into per-core input maps (host-side reshapes only)."""
    pp_all = np.asarray(positive_prob, dtype=np.float32)
    tg_all = np.asarray(target).astype(np.int64)
    pr_all = np.asarray(predictions, dtype=np.float32)
    in_maps = []
    for i in range(NCORES):
        sl = slice(i * BS, (i + 1) * BS)
        # [BS] -> [P, T]: row g*P + p lands at [p, g], matching the row tiling
        pp = np.ascontiguousarray(pp_all[sl].reshape(T, P).T)
        tg = tg_all[sl]
        tgf = np.ascontiguousarray(tg.astype(np.float32).reshape(T, P).T)
        y = np.maximum(tg - 1, 0)
        ymod = np.ascontiguousarray((y & 127).astype(np.float32).reshape(T, P).T)
        # chunk index for row r: 16*r + (y>>7), max 2047*16+15 = 32767 (int16)
        r = np.arange(BS, dtype=np.int64)
        idxf = (r * 16 + (y >> 7)).astype(np.int16)
        # dma_gather reads idx for slot i at [i % 16, i // 16] of the first 16
        # partitions; replicate the block for the 8 gpsimd cores.
        blk = np.ascontiguousarray(idxf.reshape(CHUNK, 16).T)
        idxs = np.tile(blk, (8, 1))
        pr16 = np.ascontiguousarray(pr_all[sl].astype(np.float16).reshape(T, P, C))
        in_maps.append(
            {"pred": pr16, "pprob": pp, "tgt": tgf, "ymod": ymod, "idxs": idxs}
        )
    return in_maps


_NC_CACHE = []


def kernel(positive_prob, predictions, target):
    in_maps = make_in_maps(positive_prob, predictions, target)
    if not _NC_CACHE:
        _NC_CACHE.append(build_nc())
    nc = _NC_CACHE[0]
    res = run_bass_kernel_spmd(nc, in_maps, list(range(NCORES)))
    total = np.float32(0.0)
    for r in res.results:
        total += np.float32(r["out"][0, 0])
    return np.asarray(total, dtype=np.float32)


# revision 6
# speedup vs baseline: 1.6204x; 1.6204x over previous
"""Trainium2 Bass kernel for CustomCombinedLoss (weighted BCE sum + MultiMarginLoss).

loss = -sum(w * (pos_t*log(p) + (1-pos_t)*log(1-p)))          # w=2 for target==0
     + sum_{i: target_i>0} (1/C) * sum_{j != y_i} max(0, margin - x[i,y_i] + x[i,j])

Sharding: pure data parallel over the batch dim, B=16384 rows -> 8 cores x 2048 rows.
Each core computes a partial scalar loss; host sums the 8 partials.

Key optimizations over the f32 baseline (61 us):
  - predictions are downcast to fp16 on the host: halves HBM traffic (the
    bottleneck).  Margin-term error from fp16 quantization is ~1e-6 relative,
    far inside the 2e-2 gate; the BCE side stays f32 end to end.
  - xy = pred[r, y_r] extraction no longer burns a full-tile DVE pass per tile
    (2.2 us each).  Per tile, a gpsimd ap_gather (~0.4 us, otherwise-idle
    engine) fetches 16 candidate f16 pairs per partition (each 16-partition
    group shares its rows' y>>1 indices); a [128,32] scalar_tensor_tensor
    picks the right pair slot + parity (~0.3 us on DVE).
  - the hinge is split across ACT and DVE so both engines finish with the DMA:
    ACT tiles:  activation(Relu, bias=margin-xy, accum_out)    ~2.3 us eff
    DVE tiles:  tensor_scalar(max, add) cache-reduce where
                relu(x+b) = max(x, -b) + b and the reduce seed scalar2 = C*b
                makes accum = sum_j relu(x_j + b) directly     ~2.5 us eff
    (HW: the DVE cache-reduce variant always runs 1x; 2x/4x packed modes
    exist only without accum, measured 812 ns vs 2400 ns.)
  - predictions DMA as [128, 8KB/partition] supertiles (two row tiles
    column-paired by the host) for descriptor-optimal streaming.

Layout: row r = g*128 + p of the shard lives at partition p, tile g (0..15).
pred DRAM is [NSUP=8, P=128, 2*C] fp16; supertile s holds tiles 2s, 2s+1.
"""

from contextlib import ExitStack

import numpy as np

import concourse.bacc as bacc
import concourse.bass as bass
import concourse.mybir as mybir
import concourse.tile as tile
from concourse.bass_utils import run_bass_kernel_spmd

WEIGHT = 2.0
MARGIN = 0.5
B, C = 16384, 2048
NCORES = 8
BS = B // NCORES          # rows per core
P = 128                   # partitions
T = BS // P               # row tiles per core
SUPT = 2                  # tiles per streamed supertile
NSUP = T // SUPT
NPAIR = C // 2            # f16 pairs per row
F32 = mybir.dt.float32
F16 = mybir.dt.float16
I16 = mybir.dt.int16

AluOp = mybir.AluOpType
ActFn = mybir.ActivationFunctionType
AxisList = mybir.AxisListType

# tiles whose hinge runs on DVE (rest on ACT); both engines ~23 us busy
DVE_TILES = frozenset({1, 4, 7, 9, 11, 13})


def _loss_program(nc: bass.Bass, tc: "tile.TileContext", pred, pprob, tgt, yidx,
                  posm, out):
    ctx = ExitStack()
    with ctx:
        const_pool = ctx.enter_context(tc.tile_pool(name="const", bufs=1))
        small_pool = ctx.enter_context(tc.tile_pool(name="small", bufs=1))
        pred_pool = ctx.enter_context(tc.tile_pool(name="pred", bufs=6))

        # ---- small inputs via the ACT HWDGE ring (ahead of the sync stream)
        tgt_t = small_pool.tile([P, T], F32)
        nc.scalar.dma_start(tgt_t[:], tgt[:])
        pprob_t = small_pool.tile([P, T], F32)
        nc.scalar.dma_start(pprob_t[:], pprob[:])
        yidx_t = small_pool.tile([P, T], I16)
        nc.scalar.dma_start(yidx_t[:], yidx[:])
        posm_t = small_pool.tile([P, T], F32)
        nc.scalar.dma_start(posm_t[:], posm[:])

        # iota 0..31 on every partition (f16 exact)
        iota32 = const_pool.tile([P, 32], F16)
        nc.gpsimd.iota(
            iota32[:], pattern=[[1, 32]], base=0, channel_multiplier=0,
            allow_small_or_imprecise_dtypes=True,
        )

        # scratch outputs (never read); one per engine so ACT/DVE don't
        # serialize on a shared WAW hazard
        junk_dve = const_pool.tile([P, C], F16)
        junk_act = const_pool.tile([P, C], F16)
        junk32 = const_pool.tile([P, 32], F16)

        # gathered candidate pairs, [128, 16 slots * 2] per tile
        apo = const_pool.tile([P, T * 32], F16)
        # per-row hinge params; filled per supertile as xy becomes known
        xy_t = small_pool.tile([P, T], F32)
        negb_t = small_pool.tile([P, T], F32)   # xy - margin          (DVE s1)
        cb_t = small_pool.tile([P, T], F32)     # C*(margin - xy)      (DVE s2)
        bias_t = small_pool.tile([P, T], F32)   # margin - xy          (ACT bias)
        acc_t = small_pool.tile([P, T], F32)    # sum_j relu(x_j + bias), j==y incl.

        # ---- BCE row terms (all [P, T] f32, off the critical path)
        pos_t = small_pool.tile([P, T], F32)
        nc.vector.tensor_scalar(pos_t[:], tgt_t[:], 1.0, None, AluOp.min)
        q_t = small_pool.tile([P, T], F32)
        nc.vector.tensor_scalar(q_t[:], pprob_t[:], -1.0, 1.0, AluOp.mult, AluOp.add)
        lp_t = small_pool.tile([P, T], F32)
        nc.scalar.activation(lp_t[:], pprob_t[:], ActFn.Ln)
        lq_t = small_pool.tile([P, T], F32)
        nc.scalar.activation(lq_t[:], q_t[:], ActFn.Ln)
        nc.vector.tensor_scalar(lp_t[:], lp_t[:], -100.0, None, AluOp.max)
        nc.vector.tensor_scalar(lq_t[:], lq_t[:], -100.0, None, AluOp.max)

        # row_total = pos_t*(acc/C - lp - MARGIN/C) + (2*pos_t - 2)*lq
        lp2_t = small_pool.tile([P, T], F32)
        nc.vector.tensor_scalar(lp2_t[:], lp_t[:], MARGIN / C, None, AluOp.add)
        c2_t = small_pool.tile([P, T], F32)
        nc.vector.tensor_scalar(c2_t[:], pos_t[:], 2.0, -2.0, AluOp.mult, AluOp.add)
        d_t = small_pool.tile([P, T], F32)
        nc.vector.tensor_mul(d_t[:], c2_t[:], lq_t[:])
        inv_c_t = small_pool.tile([P, 1], F32)
        nc.vector.memset(inv_c_t[:], 1.0 / C)
        ones_t = small_pool.tile([P, 1], F32)
        nc.vector.memset(ones_t[:], 1.0)

        # ---- stream supertiles; per tile: gather xy, then hinge on ACT or DVE
        for s in range(NSUP):
            st = pred_pool.tile([P, SUPT * C], F16, tag="pred")
            nc.sync.dma_start(st[:], pred[s])
            g0 = s * SUPT
            for b in range(SUPT):
                g = g0 + b
                blk = st[:, b * C : (b + 1) * C]
                # 16 candidate pairs/partition (group-shared y>>1 indices)
                pairs = blk.rearrange("p (n d) -> p n d", d=2)
                nc.gpsimd.ap_gather(
                    apo[:, g * 32 : (g + 1) * 32], pairs,
                    yidx_t[:, g : g + 1], channels=P, num_elems=NPAIR, d=2,
                    num_idxs=16,
                )
                # xy = pair value at slot (p%16)*2 + (y&1)
                nc.vector.scalar_tensor_tensor(
                    junk32[:], iota32[:], posm_t[:, g : g + 1],
                    apo[:, g * 32 : (g + 1) * 32],
                    AluOp.is_equal, AluOp.mult, accum_out=xy_t[:, g : g + 1],
                )
            cols = slice(g0, g0 + SUPT)
            nc.vector.tensor_scalar(
                negb_t[:, cols], xy_t[:, cols], -MARGIN, None, AluOp.add
            )
            nc.vector.tensor_scalar(
                cb_t[:, cols], negb_t[:, cols], -float(C), None, AluOp.mult
            )
            nc.vector.tensor_scalar(
                bias_t[:, cols], negb_t[:, cols], -1.0, None, AluOp.mult
            )
            for b in range(SUPT):
                g = g0 + b
                blk = st[:, b * C : (b + 1) * C]
                if g in DVE_TILES:
                    nc.vector.tensor_scalar(
                        junk_dve[:], blk, negb_t[:, g : g + 1],
                        cb_t[:, g : g + 1], AluOp.max, AluOp.add,
                        accum_out=acc_t[:, g : g + 1],
                    )
                else:
                    nc.scalar.activation(
                        junk_act[:], blk, ActFn.Relu, bias=bias_t[:, g : g + 1],
                        scale=1.0, accum_out=acc_t[:, g : g + 1],
                    )

        # ---- epilogue: a = acc/C - lp2;  rowred = sum_g(pos_t*a + d)
        rowred = small_pool.tile([P, 1], F32)
        a_t = small_pool.tile([P, T], F32)
        nc.vector.scalar_tensor_tensor(
            a_t[:], acc_t[:], inv_c_t[:, 0:1], lp2_t[:],
            AluOp.mult, AluOp.subtract,
        )
        b_t = small_pool.tile([P, T], F32)
        nc.vector.tensor_mul(b_t[:], pos_t[:], a_t[:])
        e_t = small_pool.tile([P, T], F32)
        nc.vector.tensor_add(e_t[:], b_t[:], d_t[:])
        nc.vector.reduce_sum(rowred[:], e_t[:], axis=AxisList.X)
        # cross-partition sum via PE: ones[128,1].T @ rowred[128,1] -> [1,1]
        psum_pool = ctx.enter_context(tc.tile_pool(name="psum", bufs=1, space="PSUM"))
        total_ps = psum_pool.tile([1, 1], F32)
        nc.tensor.matmul(total_ps[:], rowred[:], ones_t[:], start=True, stop=True)
        total = small_pool.tile([1, 1], F32)
        nc.vector.tensor_copy(total[:], total_ps[:])
        nc.sync.dma_start(out[:], total[:])


def build_nc() -> bass.Bass:
    nc = bacc.Bacc("TRN2", target_bir_lowering=False, debug=False, num_devices=NCORES)
    pred = nc.dram_tensor("pred", [NSUP, P, SUPT * C], F16, kind="ExternalInput").ap()
    pprob = nc.dram_tensor("pprob", [P, T], F32, kind="ExternalInput").ap()
    tgt = nc.dram_tensor("tgt", [P, T], F32, kind="ExternalInput").ap()
    yidx = nc.dram_tensor("yidx", [P, T], I16, kind="ExternalInput").ap()
    posm = nc.dram_tensor("posm", [P, T], F32, kind="ExternalInput").ap()
    out = nc.dram_tensor("out", [1, 1], F32, kind="ExternalOutput").ap()
    with tile.TileContext(nc) as tc:
        _loss_program(nc, tc, pred, pprob, tgt, yidx, posm, out)
    nc.compile()
    return nc


def make_in_maps(positive_prob, predictions, target):
    """Shard full inputs into per-core input maps (host-side reshapes only)."""
    pp_all = np.asarray(positive_prob, dtype=np.float32)
    tg_all = np.asarray(target).astype(np.int64)
    pr_all = np.asarray(predictions, dtype=np.float32)
    prow = np.arange(P, dtype=np.int64) % 16
    in_maps = []
    for i in range(NCORES):
        sl = slice(i * BS, (i + 1) * BS)
        # [BS] -> [P, T]: row g*P + p lands at [p, g], matching the row tiling
        pp = np.ascontiguousarray(pp_all[sl].reshape(T, P).T)
        tg = tg_all[sl]
        tgf = np.ascontiguousarray(tg.astype(np.float32).reshape(T, P).T)
        y = np.maximum(tg - 1, 0)
        # per-row pair index (y>>1) and STT compare slot (p%16)*2 + (y&1)
        yidx = np.ascontiguousarray((y >> 1).astype(np.int16).reshape(T, P).T)
        ymod2 = (y & 1).reshape(T, P).T          # [P, T]
        posm = np.ascontiguousarray(
            (prow[:, None] * 2 + ymod2).astype(np.float32)
        )
        # supertile layout: [NSUP, P, 2*C], tiles 2s,2s+1 column-paired so each
        # partition row is 8KB contiguous (descriptor-optimal DMA)
        pr16 = pr_all[sl].astype(np.float16).reshape(NSUP, SUPT, P, C)
        pr16 = np.ascontiguousarray(pr16.transpose(0, 2, 1, 3).reshape(NSUP, P, SUPT * C))
        in_maps.append(
            {"pred": pr16, "pprob": pp, "tgt": tgf, "yidx": yidx, "posm": posm}
        )
    return in_maps


_NC_CACHE = []


def kernel(positive_prob, predictions, target):
    in_maps = make_in_maps(positive_prob, predictions, target)
    if not _NC_CACHE:
        _NC_CACHE.append(build_nc())
    nc = _NC_CACHE[0]
    res = run_bass_kernel_spmd(nc, in_maps, list(range(NCORES)))
    total = np.float32(0.0)
    for r in res.results:
        total += np.float32(r["out"][0, 0])
    return np.asarray(total, dtype=np.float32)
